# revision 25
# baseline (speedup 1.0000x reference)
"""DD-RoPE kernel for 8x TRN2 NeuronCores — scan-pipelined [p,t] design (v5).

Reference computation (B=4, T=4096, D=2048, P=256):
    deltas = einsum('btd,pd->btp', x, W) + b     # (B, T, P)
    angles = cumsum(deltas, axis=1)
    out = concat([x1*cos(a) - x2*sin(a), x2*cos(a) + x1*sin(a), x[..., 512:]], -1)

Sharding: 8 shards = 4 batches x 2 T-halves (2048 steps each), data-parallel.
The cumsum is split into independent 128-step blocks: the host computes the
exact (fp64) cumulative angle at each block boundary and the device runs a
per-block prefix scan seeded with that base, so per-step rounding only
drifts within one block and no cross-core communication is needed.

Architecture (what the ~71us earlier versions got wrong):
  - The TRN2 PE p-state ramp (2.4 GHz only after ~3us of continuous busy,
    reset on idle) makes ANY PE bubble cost double. Earlier versions ran
    cumsum matmuls + transposes on the PE, forcing two PE<->Scalar/DVE
    round-trips per tile; the in-order PE queue stalled ~3us per pair and
    p-state resets doubled matmul times.
  - v5 keeps the PE stream PURE: only the 256 delta matmuls (plus a few
    p-state warm-up dummies). Everything else leaves the PE:
      * deltas are computed directly in [p, t] layout (W chunks stationary,
        x^T chunks moving) — 32 matmuls x 256 cols per block-pair;
      * the per-block cumsum is a DVE tensor_tensor_scan reading the PSUM
        deltas directly (fp32 state, per-128-step runs), with the exact
        fp32 block base as the scan initial and +b folded in via the
        second scan operand — no U-matmul, no affine matmul, no fp16
        delta copy;
      * sin/cos run on ScalarE from the range-reduced angles
        (magic-number reduction; cos = sin(pi/2 - 2pi*|rs|) with |rs| on
        ScalarE Abs — same activation-table set as Sin);
      * the rotation is elementwise in [p, t]: x1^T/x2^T are exactly the
        dc 0..3 chunks of the xt tiles already resident for the matmuls
        (the x12 input stream of v1/v2 is deleted: 2 MiB/core less DMA);
        two of the six rotation ops run on the otherwise-idle GpSimd.
  - DMA: xt (8 MiB/core) + w (1 MiB) in, out (2 MiB) out = 11 MiB at
    ~360 GB/s ~= 31us, overlapping the 28us PE stream. First pair's xt is
    split in quarters (dc-major layout) so the first matmul starts after
    ~300 KiB; dummy matmuls ramp the PE p-state while DMA streams; the
    Sin table load happens in the prologue via a dummy Sin.

Per-pair budget: PE 3.5us, DVE (4 scans + range-reduce + 4 rot) 3.4us,
Scalar (reduce/abs/sin/cos) 2.8us, GpSimd (2 rot muls + out DMA) 2.7us,
DMA 1.25 MiB = 3.6us.
"""

import sys

if "/opt/trn_rl_repo" not in sys.path:
    sys.path.insert(0, "/opt/trn_rl_repo")

from contextlib import ExitStack

import numpy as np

import concourse.bacc as bacc
import concourse.bass as bass
import concourse.mybir as mybir
import concourse.tile as tile
from concourse.bass_utils import run_bass_kernel_spmd

F32 = mybir.dt.float32
F16 = mybir.dt.float16
ADD = mybir.AluOpType.add
SUB = mybir.AluOpType.subtract
IDENT = mybir.ActivationFunctionType.Identity
SIN = mybir.ActivationFunctionType.Sin
ABS = mybir.ActivationFunctionType.Abs

D = 2048          # input feature dim (contraction)
P = 256           # delta-pairs dim
ROT = 2 * P       # rotated columns (512)
TL = 2048         # time steps per shard
BK = 128          # cumsum block (base injection granularity)
NBK = TL // BK    # blocks per shard (16)
KC = D // 128     # contraction chunks (16)
NPAIR = NBK // 2  # row-pairs in the xt dram layout (8)
N_CORES = 8

# pipeline items: (block offset, blocks in item) — tapered single-block
# tail; the last two items share pair 7's xt tile
ITEMS = [(0, 2), (2, 2), (4, 2), (6, 2), (8, 2), (10, 2), (12, 2),
         (14, 1), (15, 1)]
ITEM_OFF = []
_off = 0
for _bo, _nb in ITEMS:
    ITEM_OFF.append(_off)
    _off += 4 * _nb * 128
OUT_COLS = _off   # 8192

N_WARM_MM = 10    # dummy matmuls to ramp the PE p-state during the prologue

MAGIC = 12582912.0          # 1.5 * 2**23: fp32 round-to-int magic constant
SCALE_2PI = 6.28310         # slightly < 2*pi so Sin args stay inside [-pi, pi]
HALF_PI = 1.5707964


def build_program() -> bass.Bass:
    nc = bacc.Bacc("TRN2", target_bir_lowering=False, debug=False)

    # x^T tiles, dc-major: [r*128 + dp, (dc*2 + bkl)*128 + tl]
    #   = xs[(2r+bkl)*128 + tl, dc*128 + dp]
    xt = nc.dram_tensor("xt", [NPAIR * 128, 2 * KC * 128], F16,
                        kind="ExternalInput").ap()
    # W^T chunks for stationary use: [128 d-part, (dc*2 + pc)*128 + pj]
    #   = (W.T/2pi)[dc*128 + dpart, pc*128 + pj]
    w2 = nc.dram_tensor("w2", [128, 2 * KC * 128], F16,
                        kind="ExternalInput").ap()
    # exact fp32 block bases in [p] layout: [pp, bk*2 + pc]
    bst = nc.dram_tensor("bst", [128, NBK * 2], F32,
                         kind="ExternalInput").ap()
    # b (turns) broadcast along t: [pp, pc*128 + j] = b[pc*128+pp] for all j
    bwide = nc.dram_tensor("bwide", [128, 2 * 128], F16,
                           kind="ExternalInput").ap()
    # rotated output in [p, t] layout:
    #   [q, item_off + ((h*2 + c)*nb + b)*128 + t]
    # (q = p%128, c = p//128, h = rotation half, b = block-in-item, t local)
    outT = nc.dram_tensor("outT", [128, OUT_COLS], F16,
                          kind="ExternalOutput").ap()

    with tile.TileContext(nc) as tc, ExitStack() as ctx:
        const_pool = ctx.enter_context(tc.tile_pool(name="const", bufs=1))
        w_pool = ctx.enter_context(tc.tile_pool(name="w", bufs=1))
        xt_pool = ctx.enter_context(tc.tile_pool(name="xt", bufs=6))
        dp_pool = ctx.enter_context(
            tc.tile_pool(name="dp_psum", bufs=4, space="PSUM"))
        junk_pool = ctx.enter_context(
            tc.tile_pool(name="junk_psum", bufs=1, space="PSUM"))
        ang_pool = ctx.enter_context(tc.tile_pool(name="ang", bufs=3))
        a32_pool = ctx.enter_context(tc.tile_pool(name="a32", bufs=3))
        trig_pool = ctx.enter_context(tc.tile_pool(name="trig", bufs=3))
        rot_pool = ctx.enter_context(tc.tile_pool(name="rot", bufs=3))
        out_pool = ctx.enter_context(tc.tile_pool(name="out", bufs=3))

        # --- prologue ----------------------------------------------------
        # junk memset + dummy matmuls first (DVE-fed memset, so the PE
        # warm-up only waits on the cheap DVE queue)
        junk_sb = const_pool.tile([128, 512], F16, tag="junk")
        nc.vector.memset(junk_sb[:], 0.0)
        junk_ps = junk_pool.tile([128, 512], F32, tag="junkp")
        for _ in range(N_WARM_MM):
            nc.tensor.matmul(junk_ps[:], junk_sb[:, 0:128], junk_sb[:],
                             start=True, stop=True)

        # critical first transfers on the Activation queue, parallel to
        # SP's preamble: first w2 quarter (dc 0..3, both p-chunks)
        w_sb = w_pool.tile([128, 2 * KC * 128], F16, tag="w")
        nc.scalar.dma_start(w_sb[:, 0:1024], w2[:, 0:1024])

        bst_sb = const_pool.tile([128, NBK * 2], F32, tag="bst")
        bw_sb = const_pool.tile([128, 2 * 128], F16, tag="bwide")
        magic_sb = const_pool.tile([128, 1], F32, tag="magic")
        nc.gpsimd.memset(magic_sb[:], MAGIC)
        hpi_sb = const_pool.tile([128, 1], F32, tag="hpi")
        nc.gpsimd.memset(hpi_sb[:], HALF_PI)
        # dummy Sin pulls the 1.3us ACT_TABLE_LOAD into the prologue
        warm_sb = const_pool.tile([128, 1], F16, tag="warm")
        nc.gpsimd.memset(warm_sb[:], 0.0)
        warm2_sb = const_pool.tile([128, 1], F16, tag="warm2")
        nc.scalar.activation(warm2_sb[:], warm_sb[:], SIN)

        xtg_shared = {}

        def issue_in_dmas(it):
            bo, nb = ITEMS[it]
            r = bo // 2
            rows = slice(r * 128, (r + 1) * 128)
            if nb == 1:
                # the two tail blocks share pair 7's tile, loaded at the
                # first tail item
                if r in xtg_shared:
                    return xtg_shared[r]
                xtg = xt_pool.tile([128, 2 * KC * 128], F16, tag="xt")
                nc.sync.dma_start(xtg[:], xt[rows, :])
                xtg_shared[r] = xtg
                return xtg
            xtg = xt_pool.tile([128, 2 * KC * 128], F16, tag="xt")
            if it == 0:
                # quarters (dc-major!) so the PE starts after ~300 KiB and
                # streams: each quarter feeds 8 matmuls (0.9us) and lands
                # in 0.75us
                for q in range(4):
                    nc.scalar.dma_start(xtg[:, q * 1024:(q + 1) * 1024],
                                        xt[rows, q * 1024:(q + 1) * 1024])
            else:
                nc.sync.dma_start(xtg[:], xt[rows, :])
            return xtg

        def stage_deltas(it, xtg):
            """Delta matmuls in [p, t]: dpT[pc*nb*128 + b*128 + t]."""
            bo, nb = ITEMS[it]
            lo = bo % 2
            wid = nb * 128
            # pc OUTER: the PE cannot interleave two PSUM accumulation
            # groups (probe-verified), so each pc's dc-accumulation must be
            # a contiguous run of matmuls
            dp = dp_pool.tile([128, 2 * wid], F32, tag="dp")
            for pc in range(2):
                for dc in range(KC):
                    if nb == 2:
                        mov = xtg[:, dc * 256:(dc + 1) * 256]
                    else:
                        mov = xtg[:, dc * 256 + lo * 128:
                                   dc * 256 + (lo + 1) * 128]
                    nc.tensor.matmul(
                        dp[:, pc * wid:(pc + 1) * wid],
                        w_sb[:, (dc * 2 + pc) * 128:(dc * 2 + pc + 1) * 128],
                        mov,
                        start=(dc == 0), stop=(dc == KC - 1))
            return dp

        def stage_back(it, dp, xtg):
            """Scans + trig + [p,t] rotation + out DMA for item `it`."""
            bo, nb = ITEMS[it]
            lo = bo % 2
            wid = nb * 128
            wid2 = 2 * wid

            # per-(pc, block) prefix scan: state = (delta + state) + b,
            # seeded with the exact fp32 block base
            ang = ang_pool.tile([128, wid2], F32, tag="ang")
            for pc in range(2):
                for bkl in range(nb):
                    bk = bo + bkl
                    s = slice(pc * wid + bkl * 128, pc * wid + (bkl + 1) * 128)
                    nc.vector.tensor_tensor_scan(
                        ang[:, s], dp[:, s],
                        bw_sb[:, pc * 128:(pc + 1) * 128],
                        bst_sb[:, bk * 2 + pc:bk * 2 + pc + 1],
                        op0=ADD, op1=ADD)

            # range reduction (turns): rs = y - round(y) in [-0.5, 0.5]
            a_s = a32_pool.tile([128, wid2], F32, tag="a_s")
            nc.scalar.activation(a_s[:], ang[:], IDENT,
                                 bias=magic_sb[:], scale=-1.0)
            rs = trig_pool.tile([128, wid2], F16, tag="rs")
            nc.vector.scalar_tensor_tensor(rs[:], a_s[:], MAGIC, ang[:],
                                           op0=SUB, op1=ADD)
            sn = trig_pool.tile([128, wid2], F16, tag="sn")
            nc.scalar.activation(sn[:], rs[:], SIN, scale=SCALE_2PI)
            # cos(2pi*y) = sin(pi/2 - 2pi*|rs|)
            ra = trig_pool.tile([128, wid2], F16, tag="ra")
            nc.scalar.activation(ra[:], rs[:], ABS)
            cs = trig_pool.tile([128, wid2], F16, tag="cs")
            nc.scalar.activation(cs[:], ra[:], SIN,
                                 scale=-SCALE_2PI, bias=hpi_sb[:])

            # rotation in [p, t]; all views are (c, b, t)-ordered
            xv = xtg[:].rearrange("q (k b t) -> q k b t", k=KC, b=2, t=128)
            if nb == 2:
                x1 = xv[:, 0:2, :, :]
                x2 = xv[:, 2:4, :, :]
            else:
                x1 = xv[:, 0:2, lo:lo + 1, :]
                x2 = xv[:, 2:4, lo:lo + 1, :]
            snv = sn[:].rearrange("q (c b t) -> q c b t", c=2, b=nb, t=128)
            csv = cs[:].rearrange("q (c b t) -> q c b t", c=2, b=nb, t=128)
            o = out_pool.tile([128, 2 * wid2], F16, tag="o")
            o1 = o[:, 0:wid2].rearrange("q (c b t) -> q c b t",
                                        c=2, b=nb, t=128)
            o2 = o[:, wid2:2 * wid2].rearrange("q (c b t) -> q c b t",
                                               c=2, b=nb, t=128)

            def rv(tag):
                tl = rot_pool.tile([128, wid2], F16, tag=tag)
                return tl, tl[:].rearrange("q (c b t) -> q c b t",
                                           c=2, b=nb, t=128)

            t1, t1v = rv("t1")
            nc.vector.tensor_mul(t1v, x1, csv)
            t2, t2v = rv("t2")
            nc.vector.tensor_mul(t2v, x2, snv)
            nc.vector.tensor_sub(o1, t1v, t2v)
            t3, t3v = rv("t3")
            nc.vector.tensor_mul(t3v, x2, csv)
            t4, t4v = rv("t4")
            nc.vector.tensor_mul(t4v, x1, snv)
            nc.vector.tensor_add(o2, t3v, t4v)

            off = ITEM_OFF[it]
            nc.gpsimd.dma_start(outT[:, off:off + 4 * nb * 128], o[:])

        # remaining w2 quarters + scan constants on SP, behind pair-0's xt
        def issue_w_rest():
            for q in range(1, 4):
                nc.sync.dma_start(w_sb[:, q * 1024:(q + 1) * 1024],
                                  w2[:, q * 1024:(q + 1) * 1024])
            nc.sync.dma_start(bst_sb[:], bst[:])
            nc.sync.dma_start(bw_sb[:], bwide[:])

        pend = None  # (it, dp, xtg) awaiting its back stage
        for it in range(len(ITEMS)):
            xtg = issue_in_dmas(it)
            if it == 0:
                issue_w_rest()
            if pend is not None:
                stage_back(*pend)
            dp = stage_deltas(it, xtg)
            pend = (it, dp, xtg)
        stage_back(*pend)

    nc.compile()
    return nc


_NC_CACHE: dict = {}


def _get_nc():
    if "nc" not in _NC_CACHE:
        _NC_CACHE["nc"] = build_program()
    return _NC_CACHE["nc"]


def make_in_maps(x: np.ndarray, W: np.ndarray, b: np.ndarray):
    B, T, _ = x.shape
    inv2pi = 1.0 / (2.0 * np.pi)
    Wt = W.astype(np.float64).T * inv2pi                       # [D, P]
    wh = Wt.astype(np.float16)
    bt = b.astype(np.float64) * inv2pi                         # [P]
    # w2: [dpart, (dc*2 + pc)*128 + pj] = wh[dc*128 + dpart, pc*128 + pj]
    w2_in = np.ascontiguousarray(
        wh.reshape(KC, 128, 2, 128).transpose(1, 0, 2, 3)
        .reshape(128, 2 * KC * 128))
    # bwide: [pp, pc*128 + j] = b[pc*128 + pp]
    bw_in = np.ascontiguousarray(
        np.broadcast_to(bt.astype(np.float16).reshape(2, 128, 1),
                        (2, 128, 128)).transpose(1, 0, 2)
        .reshape(128, 2 * 128))

    # fp64 cumulative angle at every 128-step boundary, per batch (turns).
    # Computed from the full-precision weights so each block restarts at
    # the reference-exact angle.
    nblk = T // BK                                              # 32
    xblk = x.reshape(B, nblk, BK, D).sum(axis=2, dtype=np.float64)
    dblk = xblk @ Wt + BK * bt                                  # [B, 32, P]
    bases = np.zeros((B, nblk, P))
    np.cumsum(dblk[:, :-1], axis=1, out=bases[:, 1:])           # exclusive

    in_maps = []
    for c in range(N_CORES):
        bb, hh = c // 2, c % 2
        xs = x[bb, hh * TL:(hh + 1) * TL, :].astype(np.float16)  # [TL, D]
        # xt: [r*128 + dp, (dc*2 + bkl)*128 + tl]
        xt_in = np.ascontiguousarray(
            xs.reshape(NPAIR, 2, BK, KC, 128).transpose(0, 4, 3, 1, 2)
            .reshape(NPAIR * 128, 2 * KC * 128))
        bs = bases[bb, hh * NBK:(hh + 1) * NBK]                 # [NBK, P]
        # bst: [pp, bk*2 + pc] = bases[bk, pc*128 + pp]
        bst_in = np.ascontiguousarray(
            bs.reshape(NBK, 2, 128).transpose(2, 0, 1)
            .reshape(128, NBK * 2).astype(np.float32))
        in_maps.append({
            "xt": xt_in, "w2": w2_in, "bst": bst_in, "bwide": bw_in,
        })
    return in_maps


def assemble_output(x: np.ndarray, results) -> np.ndarray:
    B, T, Din = x.shape
    out = np.empty((B, T, Din), np.float32)
    out[:, :, ROT:] = x[:, :, ROT:]
    for c in range(N_CORES):
        bb, hh = c // 2, c % 2
        r = results[c]["outT"]                                # [128, 8192]
        blk = np.empty((TL, ROT), np.float32)
        for it, (bo, nb) in enumerate(ITEMS):
            off = ITEM_OFF[it]
            seg = r[:, off:off + 4 * nb * 128]                # [q, h*c*b*t]
            seg = seg.reshape(128, 2, 2, nb, 128)             # q h c b t
            # rotated[(bo+b)*128 + t, h*256 + c*128 + q]
            seg = seg.transpose(3, 4, 1, 2, 0).reshape(nb * 128, ROT)
            blk[bo * 128:(bo + nb) * 128] = seg
        out[bb, hh * TL:(hh + 1) * TL, :ROT] = blk
    return out


def kernel(x: np.ndarray, W: np.ndarray, b: np.ndarray) -> np.ndarray:
    nc = _get_nc()
    in_maps = make_in_maps(x, W, b)
    res = run_bass_kernel_spmd(nc, in_maps, list(range(N_CORES)))
    return assemble_output(x, res.results)
